# revision 37
# baseline (speedup 1.0000x reference)
"""Trainium2 Bass kernel for nn_FilteringActLayer (StyleGAN3-style filtered
leaky-relu: bias + 2x zero-insert upsample FIR (separable) + leaky-relu/gain
+ separable FIR 2x downsample).

Mixed fp8-DoubleRow / bf16 design (1 sample per core, 8 cores, pure data
parallel). Host pre-adds the bias, transposes to [h, c, w], and splits x~
into fp8 hi+lo planes (x = fp8(x) + fp8(x - fp8(x))).

  Per channel c (h' = upsampled axis of length 266 = 2*128+10):
    MM1 (fp8 DoubleRow x2): A.T[w,h'] = x~_c.T @ U1.T (up-conv along H +
        transpose). The two DR k-tiles carry the x hi/lo planes; the two
        accumulated instructions carry the U1 hi/lo fp8 split, so the
        result is bf16-accurate at half the bf16 row cost.
    MM2 (bf16 x3): B[w'_tile, h'] for w' tiles {0:128,128:256,256:266}
        from A evicted as bf16 (A must stay accurate: its fp8 noise is
        only attenuated ~0.7x per later stage, not enough for the error
        budget). The 10-row tail tile of 3 consecutive channels shares one
        PSUM bank at partition offsets {0,32,64} (plain matmuls only:
        DoubleRow requires output partition 0).
    ACT: paired Prelu eviction psum->SBUF fp8, two channels per call
        (gain folded into the MM3 weights).
    MM3 (fp8 DR x9, data-stationary): C.T[h'_tile, w''] = sum_{w'}
        B[w',h'_t] * DnW[w'',w'] -- emits the transposed down-W result
        directly (no separate transpose stage). DnW rides as fp8 hi+lo
        pairs; sigma(B) rides as raw fp8 (~1.3% noise, attenuated by the
        two remaining FIR stages).
    MM4 (bf16 x3): y[h'',w''] = sum_{h'} DnH[h'',h'] C.T[h',w'']
  Output y leaves via SBUF as bf16 (channel-paired 512B DMA runs), host
  unpacks to f32 [c, h, w].

  DoubleRow ISA constraints honored: dst partition 0, operand outer free
  steps 16B-aligned (hence the 272-col padded fp8 tiles). GPSIMD cannot
  read PSUM, so all evictions are on DVE (A, C.T, y) and Act (sigma).
"""

import numpy as np
import ml_dtypes

UP = 2
PAD_LO, PAD_HI = 11, 10
TAPS = 12
N_CORES = 8
C, H, W = 128, 128, 128
P = 128
HP = 266          # upsampled axis length
G = 8             # channels per DMA group
NG = C // G
TAIL = HP - 256   # 10

F8 = ml_dtypes.float8_e4m3fn
BF16 = ml_dtypes.bfloat16

_CACHE = {}


def _build_u1(up_filter):
    """U1 [266, 128]: up-conv matrix (zero-insert by 2, pad 11/11, taps 12)
    with the reference's per-pass gain of `up` folded in."""
    fu2 = np.asarray(up_filter, np.float64) * UP
    o = np.arange(HP)[:, None]
    j = np.arange(H)[None, :]
    t = o - 2 * j
    return np.where((t >= 0) & (t < TAPS), fu2[np.clip(t, 0, TAPS - 1)], 0.0)


def _build_dn(down_filter):
    """Dn [128, 266]: down-conv matrix (stride 2, true conv -> flipped taps)."""
    fd = np.asarray(down_filter, np.float64)
    m = np.arange(H)[:, None]
    q = np.arange(HP)[None, :]
    t = q - 2 * m
    return np.where((t >= 0) & (t < TAPS), fd[::-1][np.clip(t, 0, TAPS - 1)], 0.0)


def _hilo(w64):
    """Split a float64 matrix into fp8 hi + fp8 lo with hi+lo ~= w."""
    hi = w64.astype(F8)
    lo = (w64 - hi.astype(np.float64)).astype(F8)
    return hi, lo


def _pair(a, b):
    """Stack two [K, M] arrays into the DoubleRow [K, 2, M] k-tile layout."""
    return np.ascontiguousarray(np.stack([a, b], axis=1))


def _build_bass(slope, do_clamp, clamp, debug=False):
    import concourse.bacc as bacc
    import concourse.mybir as mybir
    from concourse import tile

    f32 = mybir.dt.float32
    bf16 = mybir.dt.bfloat16
    fp8 = mybir.dt.float8e4
    AF = mybir.ActivationFunctionType
    ALU = mybir.AluOpType
    PM = mybir.MatmulPerfMode

    nc = bacc.Bacc(None, target_bir_lowering=False, debug=False)

    # DRAM I/O.  x packed [h, hi/lo, c/4, 4w] fp8; y [h'', c/2, 2w''] bf16.
    x_d = nc.dram_tensor("x", [P, 2, C // 4, 4 * W], fp8,
                        kind="ExternalInput")
    u1p_d = nc.dram_tensor("u1p", [P, 2, 272], fp8, kind="ExternalInput")
    u2b_d = nc.dram_tensor("u2b", [P, 2, P], bf16, kind="ExternalInput")
    u2tb_d = nc.dram_tensor("u2tb", [P, TAIL], bf16, kind="ExternalInput")
    dwp_d = nc.dram_tensor("dwp", [P, 2, 2, P], fp8, kind="ExternalInput")
    dwpt_d = nc.dram_tensor("dwpt", [64 + TAIL, 2, P], fp8,
                            kind="ExternalInput")
    dh01_d = nc.dram_tensor("dh01", [P, 2, P], bf16, kind="ExternalInput")
    dht_d = nc.dram_tensor("dht", [TAIL, P], bf16, kind="ExternalInput")
    y_d = nc.dram_tensor("y", [P, C // 2, 2 * W], bf16, kind="ExternalOutput")
    if debug:
        dbg_a = nc.dram_tensor("dbg_a", [P, HP], bf16, kind="ExternalOutput")
        dbg_sb = nc.dram_tensor("dbg_sb", [P, 2, HP], fp8,
                                kind="ExternalOutput")
        dbg_sbt = nc.dram_tensor("dbg_sbt", [74, HP], fp8,
                                 kind="ExternalOutput")
        dbg_ct = nc.dram_tensor("dbg_ct", [P, 3, P], bf16,
                                kind="ExternalOutput")

    def bc2(ap, k, m):
        """[k, m] AP -> broadcast [k, 2, m] (same data in both DR k-tiles)."""
        return ap.rearrange("p (o n) -> p o n", o=1).broadcast_to([k, 2, m])

    with tile.TileContext(nc) as tc:
        with (
            tc.tile_pool(name="const", bufs=1) as const,
            tc.tile_pool(name="xb_p", bufs=3) as xb_p,
            tc.tile_pool(name="a_p", bufs=6) as a_p,
            tc.tile_pool(name="sb_p", bufs=6) as sb_p,
            tc.tile_pool(name="sbt_p", bufs=3) as sbt_p,
            tc.tile_pool(name="ct_p", bufs=4) as ct_p,
            tc.tile_pool(name="y_p", bufs=3) as y_p,
            tc.tile_pool(name="ps_bb", bufs=1, space="PSUM") as ps_bb_p,
            tc.tile_pool(name="ps_a", bufs=1, space="PSUM") as ps_a_p,
            tc.tile_pool(name="ps_t", bufs=1, space="PSUM") as ps_t_p,
            tc.tile_pool(name="ps_c", bufs=2, space="PSUM") as ps_c_p,
        ):
            u1p = const.tile([P, 2, 272], fp8)
            nc.sync.dma_start(u1p[:], u1p_d[:])
            u2b = const.tile([P, 2, P], bf16)
            nc.sync.dma_start(u2b[:], u2b_d[:])
            u2tb = const.tile([P, TAIL], bf16)
            nc.sync.dma_start(u2tb[:], u2tb_d[:])
            dwp = const.tile([P, 2, 2, P], fp8)
            nc.sync.dma_start(dwp[:], dwp_d[:])
            dwpt = const.tile([64 + TAIL, 2, P], fp8)
            nc.sync.dma_start(dwpt[:], dwpt_d[:])
            dh01 = const.tile([P, 2, P], bf16)
            nc.sync.dma_start(dh01[:], dh01_d[:])
            dht = const.tile([TAIL, P], bf16)
            nc.sync.dma_start(dht[:], dht_d[:])

            xbs, ats, sbs, sbts, cts_ps, cts, ys = {}, {}, {}, {}, {}, {}, {}
            ps_bb = ps_bb_p.tile([P, 4, 512], f32)
            ps_a = ps_a_p.tile([P, 512], f32)
            ps_t = ps_t_p.tile([P, 512], f32)

            def e_load(g):
                xb = xb_p.tile([P, 2, G, W], fp8)
                nc.sync.dma_start(
                    xb[:], x_d[:, :, 2 * g:2 * g + 2, :]
                    .rearrange("p l c (q w) -> p l (c q) w", w=W))
                xbs[g] = xb

            def e_mm1(c):
                g, ci = divmod(c, G)
                lhsT = xbs[g][:, :, ci, :]
                for j in range(2):
                    nc.tensor.matmul(
                        ps_a[:, :HP], lhsT=lhsT,
                        rhs=bc2(u1p[:, j, :HP], P, HP),
                        start=(j == 0), stop=(j == 1),
                        perf_mode=PM.DoubleRow)

            def e_a_evict(c):
                at = a_p.tile([P, HP], bf16, name="a", tag="a")
                nc.vector.tensor_copy(out=at[:], in_=ps_a[:, :HP])
                ats[c] = at
                if debug and c == 0:
                    nc.sync.dma_start(dbg_a[:], at[:])
                if c >= G:
                    xbs.pop(c // G - 1, None)

            def e_mm2_main(c):
                s0 = 2 * (c % 2)
                for j in range(2):
                    nc.tensor.matmul(
                        ps_bb[:, s0 + j, :HP], lhsT=u2b[:, j, :],
                        rhs=ats[c][:], start=True, stop=True)

            def e_mm2_tail(c):
                off = 32 * (c % 3)
                nc.tensor.matmul(ps_t[off:off + TAIL, :HP],
                                 lhsT=u2tb[:], rhs=ats[c][:],
                                 start=True, stop=True)
                ats.pop(c)

            def e_sigma_pair(c):
                # one Act call evicting both channels (c-1, c)
                sb = sb_p.tile([P, 4, 272], fp8, name="sb", tag="sb")
                nc.scalar.activation(
                    out=sb[:, :, :HP], in_=ps_bb[:, :, :HP],
                    func=AF.Prelu, bias=0.0, scale=1.0, alpha=float(slope))
                if do_clamp:
                    nc.vector.tensor_scalar(
                        out=sb[:, :, :HP], in0=sb[:, :, :HP],
                        scalar1=float(clamp), scalar2=float(-clamp),
                        op0=ALU.min, op1=ALU.max)
                if debug and c == 1:
                    nc.sync.dma_start(dbg_sb[:], sb[:, 0:2, :HP])
                sbs[c - 1] = sbs[c] = sb

            def e_sigma_tail(c):
                # after channels 3t..3t+2 wrote their tails at offsets 0/32/64
                n = c % 3 + 1
                hi = 32 * (n - 1) + TAIL
                sbt = sbt_p.tile([74, HP], fp8, name="sbt", tag="sbt")
                nc.scalar.activation(
                    out=sbt[:hi, :], in_=ps_t[:hi, :HP], func=AF.Prelu,
                    bias=0.0, scale=1.0, alpha=float(slope))
                if do_clamp:
                    nc.vector.tensor_scalar(
                        out=sbt[:hi, :], in0=sbt[:hi, :],
                        scalar1=float(clamp), scalar2=float(-clamp),
                        op0=ALU.min, op1=ALU.max)
                if debug and c == 2:
                    nc.sync.dma_start(dbg_sbt[:], sbt[:74, :])
                for cc in range(c - n + 1, c + 1):
                    sbts[cc] = sbt

            def e_mm3(c):
                sb, sbt = sbs[c], sbts[c]
                s0 = 2 * (c % 2)
                toff = 32 * (c % 3)
                ps_c = ps_c_p.tile([P, 512], f32, tag="ps_c")
                for t in range(3):
                    ts = 128 * t
                    L = P if t < 2 else TAIL
                    reg = ps_c[:L, ts:ts + P]
                    lhsT = sb[:, s0:s0 + 2, ts:ts + L]
                    nc.tensor.matmul(reg, lhsT=lhsT, rhs=dwp[:, :, 0, :],
                                     start=True, stop=False,
                                     perf_mode=PM.DoubleRow)
                    nc.tensor.matmul(reg, lhsT=lhsT, rhs=dwp[:, :, 1, :],
                                     start=False, stop=False,
                                     perf_mode=PM.DoubleRow)
                    nc.tensor.matmul(
                        reg, lhsT=bc2(sbt[toff:toff + TAIL, ts:ts + L],
                                      TAIL, L),
                        rhs=dwpt[toff:toff + TAIL, :, :],
                        start=False, stop=True, perf_mode=PM.DoubleRow)
                cts_ps[c] = ps_c
                sbs.pop(c)
                sbts.pop(c)

            def e_ct_evict(c):
                # GPSIMD cannot access PSUM (BIR verifier) -- DVE evicts
                ct = ct_p.tile([P, 3, P], bf16, name="ct", tag="ct")
                nc.vector.tensor_copy(
                    out=ct[:].rearrange("p a b -> p (a b)"),
                    in_=cts_ps[c][:, :384])
                if debug and c == 0:
                    nc.sync.dma_start(dbg_ct[:], ct[:])
                cts[c] = ct

            def e_mm4(c):
                ct = cts[c]
                ps_y = cts_ps[c]
                nc.tensor.matmul(ps_y[:, 384:512], lhsT=dh01[:, 0, :],
                                 rhs=ct[:, 0, :], start=True, stop=False)
                nc.tensor.matmul(ps_y[:, 384:512], lhsT=dh01[:, 1, :],
                                 rhs=ct[:, 1, :], start=False, stop=False)
                nc.tensor.matmul(ps_y[:, 384:512], lhsT=dht[:],
                                 rhs=ct[:TAIL, 2, :], start=False, stop=True)

            def e_y_evict(c):
                g, ci = divmod(c, G)
                if g not in ys:
                    ys[g] = y_p.tile([P, G, W], bf16, name="yo", tag="yo")
                nc.vector.tensor_copy(out=ys[g][:, ci, :],
                                      in_=cts_ps[c][:, 384:512])
                cts.pop(c)
                cts_ps.pop(c)
                if ci == G - 1:
                    nc.sync.dma_start(
                        out=y_d[:, 4 * g:4 * g + 4, :],
                        in_=ys[g][:].rearrange("p (c q) w -> p c (q w)", q=2))
                    ys.pop(g)

            # software-pipelined rounds (stage lags):
            #   c0 = r+2: load/MM1/A-evict
            #   c2 = r:   MM2-main; tail-mm + sigma for r-1;
            #             sigma-tail when (r-1)%3 == 2
            #   c3 = r-5: MM3 + CT-evict
            #   c4 = r-6: MM4 (y into the channel's ps_c bank) + y-evict
            e_load(0)
            for r in range(-2, C + 7):
                c0, c2, c3, c4 = r + 2, r, r - 5, r - 6
                cs = c2 - 1  # sigma / tail-mm channel
                if 0 <= c0 < C and c0 % G == 0 and c0 // G + 1 < NG:
                    e_load(c0 // G + 1)
                if 0 <= c2 < C:
                    e_mm2_main(c2)
                if 0 <= c3 < C:
                    e_mm3(c3)
                if 0 <= c0 < C:
                    e_mm1(c0)
                if 0 <= cs < C:
                    e_mm2_tail(cs)
                if 0 <= c4 < C:
                    e_mm4(c4)
                if 0 <= c2 < C and c2 % 2 == 1:
                    e_sigma_pair(c2)
                if 0 <= cs < C and (cs % 3 == 2 or cs == C - 1):
                    e_sigma_tail(cs)
                if 0 <= c3 < C:
                    e_ct_evict(c3)
                if 0 <= c4 < C:
                    e_y_evict(c4)
                if 0 <= c0 < C:
                    e_a_evict(c0)

    nc.compile()
    return nc


def kernel(x, b, up_filter, down_filter, gain, slope, clamp):
    from concourse.bass_utils import run_bass_kernel_spmd

    x = np.asarray(x, np.float32)
    b = np.asarray(b, np.float32)
    gain = float(np.asarray(gain))
    slope = float(np.asarray(slope))
    clamp = float(np.asarray(clamp))

    U1 = _build_u1(up_filter)          # [266, 128] f64
    DnW = _build_dn(down_filter) * gain  # gain folded into down-W weights
    DnH = _build_dn(down_filter)

    # conservative clamp-can-fire bound (matches reference data: never fires)
    l1 = float(np.abs(np.asarray(up_filter, np.float64) * UP).sum())
    xmax = float(np.abs(x + b[None, :, None, None]).max())
    do_clamp = bool(xmax * l1 * l1 * abs(gain) >= 0.98 * clamp)

    key = (round(slope, 9), do_clamp, round(clamp, 6))
    if key not in _CACHE:
        _CACHE[key] = _build_bass(slope, do_clamp, clamp)
    nc = _CACHE[key]

    # constants (hi/lo fp8 pairs)
    u1t_hi, u1t_lo = _hilo(U1.T)                       # [128, 266]
    u1p = np.zeros((P, 2, 272), F8)                    # 272 = 16-aligned pad
    u1p[:, 0, :HP] = u1t_hi
    u1p[:, 1, :HP] = u1t_lo
    u2b64 = U1[:256].T                                 # [128, 256]
    u2b = np.ascontiguousarray(
        np.stack([u2b64[:, :128], u2b64[:, 128:]], axis=1)).astype(BF16)
    u2tb = np.ascontiguousarray(U1[256:].T).astype(BF16)  # [128, 10]

    dw_hi, dw_lo = _hilo(DnW[:, :256].T)               # [256, 128]
    dwp = np.ascontiguousarray(
        np.stack([np.stack([dw_hi[:128], dw_hi[128:]], axis=1),
                  np.stack([dw_lo[:128], dw_lo[128:]], axis=1)],
                 axis=2))                              # [128, 2, 2, 128]
    # layout [p, ktile(chunk0/1), hl, :]: dwp[:, :, 0] = hi-pair, [:, :, 1] = lo
    dwt_hi, dwt_lo = _hilo(DnW[:, 256:].T)             # [10, 128]
    dwpt1 = _pair(dwt_hi, dwt_lo)                      # [10, 2, 128]
    dwpt = np.zeros((64 + TAIL, 2, P), F8)             # replicated at 0/32/64
    for off in (0, 32, 64):
        dwpt[off:off + TAIL] = dwpt1

    dh = DnH.T.astype(BF16)                            # [266, 128]
    dh01 = np.ascontiguousarray(
        np.stack([dh[:128], dh[128:256]], axis=1))     # [128, 2, 128]
    dht = np.ascontiguousarray(dh[256:])               # [10, 128]

    consts = {"u1p": u1p, "u2b": u2b, "u2tb": u2tb,
              "dwp": dwp, "dwpt": dwpt, "dh01": dh01, "dht": dht}

    xb = (x + b[None, :, None, None]).astype(np.float64)
    in_maps = []
    for n in range(N_CORES):
        xt = np.ascontiguousarray(xb[n].transpose(1, 0, 2))   # [h, c, w]
        x_hi = xt.astype(np.float32).astype(F8)
        x_lo = (xt - x_hi.astype(np.float64)).astype(np.float32).astype(F8)
        xp = np.stack([x_hi, x_lo], axis=1).reshape(P, 2, C // 4, 4 * W)
        in_maps.append({"x": xp, **consts})

    res = run_bass_kernel_spmd(nc, in_maps, core_ids=list(range(N_CORES)))
    global LAST_RESULT
    LAST_RESULT = res

    out = np.empty((N_CORES, C, H, W), np.float32)
    for n in range(N_CORES):
        yp = res.results[n]["y"].astype(np.float32)    # [128, 64, 256]
        out[n] = yp.reshape(P, C // 2, 2, W).transpose(1, 2, 0, 3) \
                   .reshape(C, H, W)
    return out


LAST_RESULT = None


if __name__ == "__main__":
    rng = np.random.default_rng(0)
    x = rng.standard_normal((N_CORES, C, H, W), np.float32)
    b = (rng.standard_normal(C) * 0.1).astype(np.float32)
    fu = rng.standard_normal(TAPS).astype(np.float32)
    fu /= np.abs(fu).sum()
    fd = rng.standard_normal(TAPS).astype(np.float32)
    fd /= np.abs(fd).sum()
    y = kernel(x, b, fu, fd, np.float32(np.sqrt(2)), np.float32(0.2),
               np.float32(256.0))
    print("kernel ran, output shape", y.shape)
